# revision 1
# baseline (speedup 1.0000x reference)
"""NeuralMMU Trainium2 kernel.

Pipeline per core (131072 addrs, 64 iterations x 2048 addrs):
  1. SP-triggered DMA of host-unpacked bit planes -> SBUF [96, 8192] u8
     (4 iters per DMA); partition q = 32s + k holds bit k (replicated 3x,
     s = 0..2), col j*2048 + 512g + c -> addr of iter j, block g
  2. DVE tensor_copy u8 -> bf16 bits [96, 2048] per iter
  3. 4x bf16 matmul k=96: bits @ (W1hi; W1mid; W1lo) -> PSUM [128,2048]
     (exact 3-way bf16 split of f32 W1, summed in the contraction dim)
  4. ACT Gelu(+b1): PSUM -> SBUF h [128,2048]
  5. 4x f32 matmul (PE col tiles 32g): h @ W2ext -> PSUM [128,512]
  6. DVE is_gt per-partition threshold (0.5 - b2): -> bf16 bits
  7. ONE bf16 matmul, block-diagonal [128,8] weights: packs all 4
     col-bands' 26 bits as lo13/hi13 in a single 512-row pass -> PSUM
  8. DVE copy PSUM -> SBUF accumulator [8,4096] (8 iters)
  9. 1x SP-triggered DMA [8,4096] per 8 iters -> DRAM;
     host combines lo + 8192*hi -> int64

The loop is software-pipelined two-deep so the PE never stalls:
PE order per iter t is L1(t+1), L2(t), pack(t-1); DVE converts bits
for t+2 while ACT runs Gelu(t) and PE runs L2(t).  This hides both
the L1(t)->Gelu(t)->L2(t) chain (Gelu finishes ~2.4us before L2
needs it) and the L2(t)->threshold(t)->pack(t) chain (threshold has
a full iteration of slack).  A small iter-0-only input DMA (R0t)
hides most of the first group-DMA latency at startup.

PE busy is ~96% of total; the f32 L2 (4 cyc/row) is optimal for the
required exactness: logit threshold gaps go down to 2.5e-8, so the
contraction must be f32-exact, and an explicit 5-pair bf16 split
would move 10240 rows/iter vs f32's effective 8192.

HW-validated: ~299 us/core, 1/1048576 mismatch (the one addr with a
2.5e-8 logit-threshold gap; same flip as a pure-f32 kernel).
"""

import numpy as np
from contextlib import ExitStack

import concourse.bass as bass
import concourse.mybir as mybir
import concourse.tile as tile
from concourse import bacc, bass_utils

B = 1_048_576
NCORES = 8
PER = B // NCORES          # 131072 addrs per core
BLK = 512                  # addrs per PE block
NBLK = 4                   # blocks per iteration
CHUNK = NBLK * BLK         # 2048 addrs per iteration
N_ITERS = PER // CHUNK     # 64
GIN = 4                    # iters per input DMA
GOUT = 8                   # iters per output DMA set

F32 = mybir.dt.float32
BF16 = mybir.dt.bfloat16
U8 = mybir.dt.uint8
AF = mybir.ActivationFunctionType
ALU = mybir.AluOpType


def build_nc(n_iters: int = N_ITERS, act=AF.Gelu) -> bass.Bass:
    nc = bacc.Bacc("TRN2")
    assert n_iters % GOUT == 0 and n_iters % GIN == 0

    bp = nc.dram_tensor("bp", [n_iters // GIN, 96, GIN * CHUNK], U8,
                        kind="ExternalInput")
    cst_d = nc.dram_tensor("cst", [128, 102], F32, kind="ExternalInput")
    outp = nc.dram_tensor("outp", [2 * NBLK, n_iters // GOUT, GOUT * BLK], F32,
                          kind="ExternalOutput")

    with ExitStack() as ctx:
        tc = ctx.enter_context(tile.TileContext(nc))
        const = ctx.enter_context(tc.tile_pool(name="const", bufs=1))
        rpool = ctx.enter_context(tc.tile_pool(name="rp", bufs=2))
        bitsp = ctx.enter_context(tc.tile_pool(name="bitsp", bufs=2))
        hp = ctx.enter_context(tc.tile_pool(name="hp", bufs=2))
        bop = ctx.enter_context(tc.tile_pool(name="bop", bufs=2))
        pksp = ctx.enter_context(tc.tile_pool(name="pksp", bufs=2))
        hprep = ctx.enter_context(tc.tile_pool(name="hprep", bufs=1, space="PSUM"))
        l2p = ctx.enter_context(tc.tile_pool(name="l2p", bufs=2, space="PSUM"))
        pkp = ctx.enter_context(tc.tile_pool(name="pkp", bufs=2, space="PSUM"))

        cst = const.tile([128, 102], F32)
        nc.sync.dma_start(cst[:], cst_d[:])
        w1b = cst[:, 0:64].bitcast(BF16)     # [128, 128] bf16; rows 0-95 used
        w2s = cst[:, 64:96]
        b1c = cst[:, 96:97]
        thc = cst[:, 97:98]
        pwc = cst[:, 98:102].bitcast(BF16)   # [128, 8] block-diag pack weights

        R = None
        pks = None

        def load_input(t):
            nonlocal R
            if t % GIN == 0:
                R = rpool.tile([96, GIN * CHUNK], U8)
                nc.sync.dma_start(R[:], bp[t // GIN])

        def convert(t):
            bits = bitsp.tile([96, CHUNK], BF16)
            nc.vector.tensor_copy(
                bits[:], R[:, CHUNK * (t % GIN):CHUNK * (t % GIN + 1)]
            )
            return bits

        def l1mm(bits):
            hpre = hprep.tile([128, CHUNK], F32)
            for g in range(NBLK):
                nc.tensor.matmul(
                    hpre[:, BLK * g:BLK * (g + 1)],
                    w1b[0:96, :],
                    bits[0:96, BLK * g:BLK * (g + 1)],
                    start=True, stop=True, tile_position=(0, 0),
                )
            return hpre

        R0t = rpool.tile([96, CHUNK], U8)
        nc.sync.dma_start(R0t[:], bp[0, :, 0:CHUNK])
        load_input(0)
        bits0 = bitsp.tile([96, CHUNK], BF16)
        nc.vector.tensor_copy(bits0[:], R0t[:])
        hpre = l1mm(bits0)
        if n_iters > 1:
            bits_next = convert(1)

        bo_prev = None

        def pack_and_store(tp):
            nonlocal pks
            pk = pkp.tile([2 * NBLK, BLK], F32)
            nc.tensor.matmul(
                pk[:],
                pwc[:],
                bo_prev[:],
                start=True, stop=True, tile_position=(0, 0),
            )
            if tp % GOUT == 0:
                pks = pksp.tile([2 * NBLK, GOUT * BLK], F32)
            nc.vector.tensor_copy(
                pks[:, BLK * (tp % GOUT):BLK * (tp % GOUT + 1)], pk[:]
            )
            if tp % GOUT == GOUT - 1:
                nc.sync.dma_start(outp[:, tp // GOUT, :], pks[:])

        for t in range(n_iters):
            h = hp.tile([128, CHUNK], F32)
            nc.scalar.activation(h[:], hpre[:], act, bias=b1c, scale=1.0)

            if t + 2 < n_iters:
                load_input(t + 2)
                bits_fut = convert(t + 2)

            if t + 1 < n_iters:
                hpre = l1mm(bits_next)
                if t + 2 < n_iters:
                    bits_next = bits_fut

            l2o = l2p.tile([128, BLK], F32)
            for g in range(NBLK):
                nc.tensor.matmul(
                    l2o[32 * g:32 * (g + 1), :],
                    w2s[:],
                    h[:, BLK * g:BLK * (g + 1)],
                    start=True, stop=True, tile_position=(0, 32 * g),
                )

            if t > 0:
                pack_and_store(t - 1)

            bo = bop.tile([128, BLK], BF16)
            nc.vector.tensor_scalar(
                bo[:], l2o[:], thc, None, op0=ALU.is_gt,
            )
            bo_prev = bo

        pack_and_store(n_iters - 1)

    return nc


def make_const_inputs(W1, b1, W2, b2):
    import ml_dtypes

    w1 = np.ascontiguousarray(W1[0:32, :], dtype=np.float32)
    hi = w1.astype(ml_dtypes.bfloat16)
    mid = (w1 - hi.astype(np.float32)).astype(ml_dtypes.bfloat16)
    lo = (w1 - hi.astype(np.float32) - mid.astype(np.float32)).astype(
        ml_dtypes.bfloat16
    )
    w1b = np.zeros((128, 128), dtype=ml_dtypes.bfloat16)
    w1b[0:32] = hi
    w1b[32:64] = mid
    w1b[64:96] = lo

    w2s = np.zeros((128, 32), dtype=np.float32)
    w2s[:, :26] = W2[:, :26]
    b1c = np.asarray(b1, dtype=np.float32).reshape(128, 1)
    thc = np.full((128, 1), 1e30, dtype=np.float32)
    pwc = np.zeros((128, 8), dtype=np.float32)
    for g in range(4):
        thc[32 * g:32 * g + 26, 0] = 0.5 - np.asarray(b2[:26], dtype=np.float32)
        for i in range(13):
            pwc[32 * g + i, 2 * g] = float(1 << i)
            pwc[32 * g + 13 + i, 2 * g + 1] = float(1 << i)
    cst = np.empty((128, 102), dtype=np.float32)
    cst[:, 0:64] = np.ascontiguousarray(w1b).view(np.float32)
    cst[:, 64:96] = w2s
    cst[:, 96:97] = b1c
    cst[:, 97:98] = thc
    cst[:, 98:102] = (
        np.ascontiguousarray(pwc.astype(ml_dtypes.bfloat16)).view(np.float32)
    )
    return {"cst": cst}


def make_bit_planes(virtual_addr, n_iters: int = N_ITERS):
    """Per-core [n_iters//GIN, 96, GIN*2048] u8 0/1 bit-plane arrays.

    Partition 32s + k (s = 0..2 replication), col j*2048 + 512g + c =
    bit k of addr (GIN*tt + j)*2048 + g*512 + c.
    """
    va32 = np.asarray(virtual_addr).astype(np.uint32)
    per = n_iters * CHUNK
    ncores = va32.size // per
    out = []
    for c in range(ncores):
        seg = va32[c * per:(c + 1) * per]
        byt = seg.view(np.uint8).reshape(n_iters // GIN, GIN, NBLK, BLK, 4)
        bits = np.unpackbits(byt, axis=-1, bitorder="little")
        # (tt, j, g, c, k) -> (tt, k, j, g, c)
        pl = bits.transpose(0, 4, 1, 2, 3).reshape(n_iters // GIN, 32, GIN * CHUNK)
        out.append(np.ascontiguousarray(np.concatenate([pl, pl, pl], axis=1)))
    return out


def combine_output(o, n_iters: int = N_ITERS):
    """[8, n_iters//GOUT, GOUT*512] f32 -> [per] int64."""
    arr = o.reshape(NBLK, 2, n_iters // GOUT, GOUT, BLK)
    lo = arr[:, 0].transpose(1, 2, 0, 3).reshape(-1).astype(np.int64)
    hi = arr[:, 1].transpose(1, 2, 0, 3).reshape(-1).astype(np.int64)
    return lo + 8192 * hi


_NC_CACHE = {}
TRACE = False
LAST_RES = None


def kernel(virtual_addr, W1, b1, W2, b2):
    global LAST_RES
    if "nc" not in _NC_CACHE:
        nc = build_nc(N_ITERS)
        nc.finalize()
        _NC_CACHE["nc"] = nc
    nc = _NC_CACHE["nc"]

    consts = make_const_inputs(W1, b1, W2, b2)
    planes = make_bit_planes(virtual_addr, N_ITERS)
    in_maps = [{"bp": planes[c], **consts} for c in range(NCORES)]

    res = bass_utils.run_bass_kernel_spmd(
        nc, in_maps, list(range(NCORES)), trace=TRACE
    )
    LAST_RES = res

    outs = [combine_output(res.results[c]["outp"]) for c in range(NCORES)]
    return np.concatenate(outs)



# revision 2
# speedup vs baseline: 1.0011x; 1.0011x over previous
"""NeuralMMU Trainium2 kernel, v2 — transposed second layer.

Per core (131072 addrs), 87 iterations sized [512, 1024, 84x1536, 512]
(small ramp-up so the first Gelu starts early; small final iteration so
the pipeline drain is short).

  1. Host sends bit planes as bf16 [96, 131072] (bit k of addr a at
     partition k, replicated 3x for the 3-way bf16 split of W1). Input
     DMA groups: single iterations during ramp-up, pairs afterwards.
  2. L1: bf16 matmuls k=96 (512-addr blocks): bits @ (W1hi;W1mid;W1lo)
     -> PSUM hpre [128, <=1536] (exact: bits are 0/1, f32 accumulate).
  3. ACT Gelu(+b1): PSUM -> SBUF h f32, one instr/iter (the modeled
     bottleneck: ~0.83 ns/elem).
  4. L2 TRANSPOSED: per 128-addr chunk, matmul with the h chunk
     [128 hid, 128 addr] as the *stationary* operand and W2[:, :26] f32
     as the *moving* operand -> PSUM pk [128 addr, nch*26] f32, exact,
     all chunks in one PSUM bank (start on first chunk, stop on last).
     Model cost 26*4 cyc/chunk vs 512*4 cyc per 512 addrs when h is the
     moving side: ~4.7x less PE time for the heavy layer.
  5. DVE: is_gt vs replicated thresholds (0.5 - b2[k]) -> bf16 bits,
     multiply by replicated 2^i weights (i = bit index within the lo/hi
     13-bit half), 4D tensor_reduce -> [128, 2*nch] f32 (lo, hi).
  6. Output batched ~8 iters per DMA; host combines lo + 8192*hi.

PSUM: hpre 2 bufs x 3 banks + pk 2 bufs x 1 bank = 8 banks exactly.
A dependency-free warm-up activation loads the Gelu table at t~0, and
the cst DMA is split so L1/Gelu constants arrive first.
"""

import numpy as np
from contextlib import ExitStack

import concourse.bass as bass
import concourse.mybir as mybir
import concourse.tile as tile
from concourse import bacc, bass_utils

B = 1_048_576
NCORES = 8
PER = B // NCORES            # 131072 addrs per core
BLK = 512                    # addrs per L1 matmul block
CHUNK = 1536                 # max addrs per iteration (3 PSUM banks)
MMCH = 128                   # addrs per transposed L2 matmul
NBITS = 26

SIZES = [512, 1024] + [1536] * 84 + [512]
assert sum(SIZES) == PER
N_ITERS = len(SIZES)         # 87
CSTART = [0]
for _s in SIZES:
    CSTART.append(CSTART[-1] + _s)

# Input DMA groups (lists of iterations): singles during ramp-up, pairs after.
GROUPS = [[0], [1], [2], [3]] + [[i, i + 1] for i in range(4, 86, 2)] + [[86]]
assert [t for g in GROUPS for t in g] == list(range(N_ITERS))
GRP_OF = {}
for _gi, _g in enumerate(GROUPS):
    for _t in _g:
        GRP_OF[_t] = _gi

# Output DMA batches: eight iterations each, then the tail alone so the
# final DMA after the last compute is tiny.
OBATCH = [list(range(r, min(r + 8, 86))) for r in range(0, 86, 8)] + [[86]]
OB_OF = {}
OB_OFF = {}
OB_USED = []
for _bi, _b in enumerate(OBATCH):
    used = 0
    for _t in _b:
        OB_OF[_t] = _bi
        OB_OFF[_t] = used
        used += 2 * (SIZES[_t] // MMCH)
    OB_USED.append(used)
NOUT = len(OBATCH)
OUTW = 24 * 8

F32 = mybir.dt.float32
BF16 = mybir.dt.bfloat16
AF = mybir.ActivationFunctionType
ALU = mybir.AluOpType
AX = mybir.AxisListType

# cst column layout (f32 columns); part A (w1b + b1) is DMA'd first so
# L1/Gelu can start before the larger part B arrives.
C_W1 = 0          # [128, 64] f32 = [128, 128] bf16 3-way W1 split
C_B1 = 64         # [128, 1] f32
C_A = 65          # end of part A
C_W2 = 65         # [128, 26] f32
C_TR = 91         # [128, 312] f32 thresholds (0.5 - b2[k]) replicated x12
C_WR = 403        # [128, 156] f32 = [128, 312] bf16 pack weights 2^i
C_TOT = 559


def build_nc() -> bass.Bass:
    nc = bacc.Bacc("TRN2")

    bp = nc.dram_tensor("bp", [96, PER], BF16, kind="ExternalInput")
    cst_d = nc.dram_tensor("cst", [128, C_TOT], F32, kind="ExternalInput")
    outp = nc.dram_tensor("outp", [NOUT, 128, OUTW], F32, kind="ExternalOutput")

    with ExitStack() as ctx:
        tc = ctx.enter_context(tile.TileContext(nc))
        const = ctx.enter_context(tc.tile_pool(name="const", bufs=1))
        rpool = ctx.enter_context(tc.tile_pool(name="rp", bufs=3))
        hpre_p = ctx.enter_context(tc.tile_pool(name="hpre", bufs=2, space="PSUM"))
        hp = ctx.enter_context(tc.tile_pool(name="hp", bufs=2))
        pkp = ctx.enter_context(tc.tile_pool(name="pkp", bufs=2, space="PSUM"))
        yp = ctx.enter_context(tc.tile_pool(name="yp", bufs=2))
        zp = ctx.enter_context(tc.tile_pool(name="zp", bufs=2))
        so_p = ctx.enter_context(tc.tile_pool(name="so", bufs=2))

        # Dependency-free warm-up: loads the Gelu table at t~0, overlapped
        # with the startup DMAs, so Gelu(0) is not blocked by it.
        warm = const.tile([128, 1], F32)
        nc.vector.memset(warm[:], 0.0)
        nc.scalar.activation(warm[:], warm[:], AF.Gelu)

        cst = const.tile([128, C_TOT], F32)
        nc.sync.dma_start(cst[:, :C_A], cst_d[:, :C_A])

        w1b = cst[:, C_W1:C_W1 + 64].bitcast(BF16)       # [128, 128] bf16
        b1c = cst[:, C_B1:C_B1 + 1]
        w2s = cst[:, C_W2:C_W2 + NBITS]                  # [128, 26] f32
        trep = cst[:, C_TR:C_TR + 312]                   # [128, 312] f32
        wrep = cst[:, C_WR:C_WR + 156].bitcast(BF16)     # [128, 312] bf16

        R = [None, None, None]
        next_group = 0

        def load_group(gi):
            g = GROUPS[gi]
            lo, hi = CSTART[g[0]], CSTART[g[-1] + 1]
            Rg = rpool.tile([96, 2 * CHUNK], BF16, name="Rg", tag="R")
            nc.sync.dma_start(Rg[:, : hi - lo], bp[:, lo:hi])
            R[gi % 3] = Rg

        def prefetch(upto_iter):
            nonlocal next_group
            while (next_group < len(GROUPS)
                   and GROUPS[next_group][0] <= upto_iter):
                load_group(next_group)
                next_group += 1

        def bits_of(t):
            gi = GRP_OF[t]
            off = CSTART[t] - CSTART[GROUPS[gi][0]]
            return R[gi % 3][0:96, off:off + SIZES[t]]

        def l1mm(t):
            na = SIZES[t]
            hpre = hpre_p.tile([128, CHUNK], F32, name="hpre", tag="hpre")
            bits = bits_of(t)
            for b in range(0, na, BLK):
                nc.tensor.matmul(
                    hpre[:, b:b + BLK],
                    w1b[0:96, :],
                    bits[:, b:b + BLK],
                    start=True, stop=True, tile_position=(0, 0),
                )
            return hpre

        # Startup: bits for iterations 0-1, then the rest of the constants,
        # then iteration 2 bits; in-loop prefetch takes over from there.
        prefetch(1)
        nc.sync.dma_start(cst[:, C_A:], cst_d[:, C_A:])
        prefetch(2)
        hpre_cur = l1mm(0)

        Sb = None
        for t in range(N_ITERS):
            na = SIZES[t]
            nch = na // MMCH

            # ACT: Gelu for iter t (waits L1(t))
            h = hp.tile([128, CHUNK], F32, name="h", tag="h")
            nc.scalar.activation(
                h[:, :na], hpre_cur[:, :na], AF.Gelu, bias=b1c, scale=1.0
            )

            # PE: L1 for iter t+1 runs while ACT does Gelu(t)
            if t + 1 < N_ITERS:
                hpre_cur = l1mm(t + 1)

            # Input prefetch ~4 iterations ahead. Issued after l1mm(t+1) so
            # the recycled R slot's readers are all already in the program.
            prefetch(t + 4)

            # PE: transposed L2 for iter t (waits Gelu(t))
            pk = pkp.tile([128, 312], F32, name="pk", tag="pk")
            for c in range(nch):
                nc.tensor.matmul(
                    pk[:, NBITS * c:NBITS * (c + 1)],
                    h[:, MMCH * c:MMCH * (c + 1)],
                    w2s[:],
                    start=(c == 0), stop=(c == nch - 1),
                )

            # DVE: threshold, weight, pack
            ncol = NBITS * nch
            Y = yp.tile([128, 312], BF16, name="Y", tag="Y")
            nc.vector.tensor_tensor(Y[:, :ncol], pk[:, :ncol], trep[:, :ncol],
                                    ALU.is_gt)
            Z = zp.tile([128, 312], BF16, name="Z", tag="Z")
            nc.vector.tensor_tensor(Z[:, :ncol], Y[:, :ncol], wrep[:, :ncol],
                                    ALU.mult)
            if OB_OFF[t] == 0:
                Sb = so_p.tile([128, OUTW], F32, name="Sb", tag="S")
            z4 = Z[:, :ncol].rearrange("p (c h b) -> p c h b", c=nch, h=2, b=13)
            off = OB_OFF[t]
            nc.vector.tensor_reduce(Sb[:, off:off + 2 * nch], z4, AX.X, ALU.add)

            bi = OB_OF[t]
            if t == OBATCH[bi][-1]:
                used = OB_USED[bi]
                nc.sync.dma_start(outp[bi][:, :used], Sb[:, :used])

    return nc


def make_const_inputs(W1, b1, W2, b2):
    import ml_dtypes

    w1 = np.ascontiguousarray(W1[0:32, :], dtype=np.float32)
    hi = w1.astype(ml_dtypes.bfloat16)
    mid = (w1 - hi.astype(np.float32)).astype(ml_dtypes.bfloat16)
    lo = (w1 - hi.astype(np.float32) - mid.astype(np.float32)).astype(
        ml_dtypes.bfloat16
    )
    w1b = np.zeros((128, 128), dtype=ml_dtypes.bfloat16)
    w1b[0:32] = hi
    w1b[32:64] = mid
    w1b[64:96] = lo

    thr = (0.5 - np.asarray(b2[:NBITS], dtype=np.float32))  # [26]
    trep = np.tile(thr, 12)[None, :].repeat(128, axis=0)    # [128, 312]

    wvec = np.zeros(312, dtype=np.float32)
    for c in range(12):
        for h in range(2):
            for i in range(13):
                wvec[26 * c + 13 * h + i] = float(1 << i)
    wrep = wvec[None, :].repeat(128, axis=0).astype(ml_dtypes.bfloat16)

    cst = np.zeros((128, C_TOT), dtype=np.float32)
    cst[:, C_W1:C_W1 + 64] = np.ascontiguousarray(w1b).view(np.float32)
    cst[:, C_B1] = np.asarray(b1, dtype=np.float32)
    cst[:, C_W2:C_W2 + NBITS] = np.asarray(W2[:, :NBITS], dtype=np.float32)
    cst[:, C_TR:C_TR + 312] = trep
    cst[:, C_WR:C_WR + 156] = np.ascontiguousarray(wrep).view(np.float32)
    return {"cst": cst}


def make_bit_planes(virtual_addr):
    """Per-core [96, PER] bf16 0/1 bit planes (3x replicated)."""
    import ml_dtypes

    va32 = np.asarray(virtual_addr).astype(np.uint32)
    out = []
    for c in range(va32.size // PER):
        seg = va32[c * PER:(c + 1) * PER]
        bits = np.unpackbits(
            seg.view(np.uint8).reshape(-1, 4), axis=-1, bitorder="little"
        )  # [PER, 32]
        u16 = (bits.T.astype(np.uint16) * 0x3F80)  # [32, PER] bf16 bit pattern
        full = np.concatenate([u16, u16, u16], axis=0)  # [96, PER]
        out.append(np.ascontiguousarray(full).view(ml_dtypes.bfloat16))
    return out


def combine_output(o):
    """[NOUT, 128, OUTW] f32 -> [PER] int64."""
    res = np.empty(PER, dtype=np.int64)
    for t in range(N_ITERS):
        nch = SIZES[t] // MMCH
        off = OB_OFF[t]
        s = o[OB_OF[t], :, off:off + 2 * nch]       # [128, 2*nch]
        lo = s[:, 0::2].astype(np.int64)            # [128, nch]
        hi = s[:, 1::2].astype(np.int64)
        phys = (lo + 8192 * hi).T.reshape(-1)       # (chunk, partition) order
        res[CSTART[t]:CSTART[t + 1]] = phys
    return res


_NC_CACHE = {}
TRACE = False
LAST_RES = None


def kernel(virtual_addr, W1, b1, W2, b2):
    global LAST_RES
    if "nc" not in _NC_CACHE:
        nc = build_nc()
        nc.finalize()
        _NC_CACHE["nc"] = nc
    nc = _NC_CACHE["nc"]

    consts = make_const_inputs(W1, b1, W2, b2)
    planes = make_bit_planes(virtual_addr)
    in_maps = [{"bp": planes[c], **consts} for c in range(NCORES)]

    res = bass_utils.run_bass_kernel_spmd(
        nc, in_maps, list(range(NCORES)), trace=TRACE
    )
    LAST_RES = res

    outs = [combine_output(res.results[c]["outp"]) for c in range(NCORES)]
    return np.concatenate(outs)


# revision 3
# speedup vs baseline: 1.0054x; 1.0043x over previous
"""NeuralMMU Trainium2 kernel, v2 — transposed second layer.

Per core (131072 addrs), 87 iterations sized [512, 1024, 84x1536, 512]
(small ramp-up so the first Gelu starts early; small final iteration so
the pipeline drain is short).

  1. Host sends bit planes as bf16 [96, 131072] (bit k of addr a at
     partition k, replicated 3x for the 3-way bf16 split of W1). Input
     DMA groups: single iterations during ramp-up, pairs afterwards.
  2. L1: bf16 matmuls k=96 (512-addr blocks): bits @ (W1hi;W1mid;W1lo)
     -> PSUM hpre [128, <=1536] (exact: bits are 0/1, f32 accumulate).
  3. ACT Gelu(+b1): PSUM -> SBUF h f32, one instr/iter (the modeled
     bottleneck: ~0.83 ns/elem).
  4. L2 TRANSPOSED: per 128-addr chunk, matmul with the h chunk
     [128 hid, 128 addr] as the *stationary* operand and W2[:, :26] f32
     as the *moving* operand -> PSUM pk [128 addr, nch*26] f32, exact,
     all chunks in one PSUM bank (start on first chunk, stop on last).
     Model cost 26*4 cyc/chunk vs 512*4 cyc per 512 addrs when h is the
     moving side: ~4.7x less PE time for the heavy layer.
  5. DVE: is_gt vs replicated thresholds (0.5 - b2[k]) -> bf16 bits,
     multiply by replicated 2^i weights (i = bit index within the lo/hi
     13-bit half), 4D tensor_reduce -> [128, 2*nch] f32 (lo, hi).
  6. Output batched ~8 iters per DMA; host combines lo + 8192*hi.

PSUM: hpre 2 bufs x 3 banks + pk 2 bufs x 1 bank = 8 banks exactly.
A dependency-free warm-up activation loads the Gelu table at t~0, and
the cst DMA is split so L1/Gelu constants arrive first.
"""

import numpy as np
from contextlib import ExitStack

import concourse.bass as bass
import concourse.mybir as mybir
import concourse.tile as tile
from concourse import bacc, bass_utils

B = 1_048_576
NCORES = 8
PER = B // NCORES            # 131072 addrs per core
BLK = 512                    # addrs per L1 matmul block
CHUNK = 1536                 # max addrs per iteration (3 PSUM banks)
MMCH = 128                   # addrs per transposed L2 matmul
NBITS = 26

SIZES = [512, 1024] + [1536] * 84 + [512]
assert sum(SIZES) == PER
N_ITERS = len(SIZES)         # 87
CSTART = [0]
for _s in SIZES:
    CSTART.append(CSTART[-1] + _s)

# Input DMA groups (lists of iterations): singles during ramp-up, pairs after.
GROUPS = [[0], [1], [2], [3]] + [[i, i + 1] for i in range(4, 86, 2)] + [[86]]
assert [t for g in GROUPS for t in g] == list(range(N_ITERS))
GRP_OF = {}
for _gi, _g in enumerate(GROUPS):
    for _t in _g:
        GRP_OF[_t] = _gi

# Output DMA batches: eight iterations each, then the tail alone so the
# final DMA after the last compute is tiny.
OBATCH = [list(range(r, min(r + 16, 86))) for r in range(0, 86, 16)] + [[86]]
OB_OF = {}
OB_OFF = {}
OB_USED = []
for _bi, _b in enumerate(OBATCH):
    used = 0
    for _t in _b:
        OB_OF[_t] = _bi
        OB_OFF[_t] = used
        used += 2 * (SIZES[_t] // MMCH)
    OB_USED.append(used)
NOUT = len(OBATCH)
OUTW = 24 * 16

F32 = mybir.dt.float32
BF16 = mybir.dt.bfloat16
AF = mybir.ActivationFunctionType
ALU = mybir.AluOpType
AX = mybir.AxisListType

# cst column layout (f32 columns); part A (w1b + b1) is DMA'd first so
# L1/Gelu can start before the larger part B arrives.
C_W1 = 0          # [128, 64] f32 = [128, 128] bf16 3-way W1 split
C_B1 = 64         # [128, 1] f32
C_A = 65          # end of part A
C_W2 = 65         # [128, 26] f32
C_TR = 91         # [128, 312] f32 thresholds (0.5 - b2[k]) replicated x12
C_WR = 403        # [128, 156] f32 = [128, 312] bf16 pack weights 2^i
C_TOT = 559


def build_nc() -> bass.Bass:
    nc = bacc.Bacc("TRN2")

    bp = nc.dram_tensor("bp", [96, PER], BF16, kind="ExternalInput")
    cst_d = nc.dram_tensor("cst", [128, C_TOT], F32, kind="ExternalInput")
    outp = nc.dram_tensor("outp", [NOUT, 128, OUTW], F32, kind="ExternalOutput")

    with ExitStack() as ctx:
        tc = ctx.enter_context(tile.TileContext(nc))
        const = ctx.enter_context(tc.tile_pool(name="const", bufs=1))
        rpool = ctx.enter_context(tc.tile_pool(name="rp", bufs=4))
        hpre_p = ctx.enter_context(tc.tile_pool(name="hpre", bufs=2, space="PSUM"))
        hp = ctx.enter_context(tc.tile_pool(name="hp", bufs=2))
        pkp = ctx.enter_context(tc.tile_pool(name="pkp", bufs=2, space="PSUM"))
        yp = ctx.enter_context(tc.tile_pool(name="yp", bufs=2))
        zp = ctx.enter_context(tc.tile_pool(name="zp", bufs=2))
        so_p = ctx.enter_context(tc.tile_pool(name="so", bufs=2))

        # Dependency-free warm-up: loads the Gelu table at t~0, overlapped
        # with the startup DMAs, so Gelu(0) is not blocked by it.
        warm = const.tile([128, 1], F32)
        nc.vector.memset(warm[:], 0.0)
        nc.scalar.activation(warm[:], warm[:], AF.Gelu)

        cst = const.tile([128, C_TOT], F32)
        nc.sync.dma_start(cst[:, :C_A], cst_d[:, :C_A])

        w1b = cst[:, C_W1:C_W1 + 64].bitcast(BF16)       # [128, 128] bf16
        b1c = cst[:, C_B1:C_B1 + 1]
        w2s = cst[:, C_W2:C_W2 + NBITS]                  # [128, 26] f32
        trep = cst[:, C_TR:C_TR + 312]                   # [128, 312] f32
        wrep = cst[:, C_WR:C_WR + 156].bitcast(BF16)     # [128, 312] bf16

        R = [None, None, None, None]
        next_group = 0

        def load_group(gi):
            g = GROUPS[gi]
            lo, hi = CSTART[g[0]], CSTART[g[-1] + 1]
            Rg = rpool.tile([96, 2 * CHUNK], BF16, name="Rg", tag="R")
            nc.sync.dma_start(Rg[:, : hi - lo], bp[:, lo:hi])
            R[gi % 4] = Rg

        def prefetch(upto_iter):
            nonlocal next_group
            while (next_group < len(GROUPS)
                   and GROUPS[next_group][0] <= upto_iter):
                load_group(next_group)
                next_group += 1

        def bits_of(t):
            gi = GRP_OF[t]
            off = CSTART[t] - CSTART[GROUPS[gi][0]]
            return R[gi % 4][0:96, off:off + SIZES[t]]

        def l1mm(t):
            na = SIZES[t]
            hpre = hpre_p.tile([128, CHUNK], F32, name="hpre", tag="hpre")
            bits = bits_of(t)
            for b in range(0, na, BLK):
                nc.tensor.matmul(
                    hpre[:, b:b + BLK],
                    w1b[0:96, :],
                    bits[:, b:b + BLK],
                    start=True, stop=True, tile_position=(0, 0),
                )
            return hpre

        # Startup: bits for iterations 0-1, then the rest of the constants,
        # then iteration 2 bits; in-loop prefetch takes over from there.
        prefetch(1)
        nc.sync.dma_start(cst[:, C_A:], cst_d[:, C_A:])
        prefetch(2)
        hpre_cur = l1mm(0)

        Sb = None
        for t in range(N_ITERS):
            na = SIZES[t]
            nch = na // MMCH

            # ACT: Gelu for iter t (waits L1(t))
            h = hp.tile([128, CHUNK], F32, name="h", tag="h")
            nc.scalar.activation(
                h[:, :na], hpre_cur[:, :na], AF.Gelu, bias=b1c, scale=1.0
            )

            # PE: L1 for iter t+1 runs while ACT does Gelu(t)
            if t + 1 < N_ITERS:
                hpre_cur = l1mm(t + 1)

            # Input prefetch ~6 iterations ahead. Issued after l1mm(t+1) so
            # the recycled R slot's readers are all already in the program.
            prefetch(t + 6)

            # PE: transposed L2 for iter t (waits Gelu(t))
            pk = pkp.tile([128, 312], F32, name="pk", tag="pk")
            for c in range(nch):
                nc.tensor.matmul(
                    pk[:, NBITS * c:NBITS * (c + 1)],
                    h[:, MMCH * c:MMCH * (c + 1)],
                    w2s[:],
                    start=(c == 0), stop=(c == nch - 1),
                )

            # DVE: threshold, weight, pack
            ncol = NBITS * nch
            Y = yp.tile([128, 312], BF16, name="Y", tag="Y")
            nc.vector.tensor_tensor(Y[:, :ncol], pk[:, :ncol], trep[:, :ncol],
                                    ALU.is_gt)
            Z = zp.tile([128, 312], BF16, name="Z", tag="Z")
            nc.vector.tensor_tensor(Z[:, :ncol], Y[:, :ncol], wrep[:, :ncol],
                                    ALU.mult)
            if OB_OFF[t] == 0:
                Sb = so_p.tile([128, OUTW], F32, name="Sb", tag="S")
            z4 = Z[:, :ncol].rearrange("p (c h b) -> p c h b", c=nch, h=2, b=13)
            off = OB_OFF[t]
            nc.vector.tensor_reduce(Sb[:, off:off + 2 * nch], z4, AX.X, ALU.add)

            bi = OB_OF[t]
            if t == OBATCH[bi][-1]:
                used = OB_USED[bi]
                nc.sync.dma_start(outp[bi][:, :used], Sb[:, :used])

    return nc


def make_const_inputs(W1, b1, W2, b2):
    import ml_dtypes

    w1 = np.ascontiguousarray(W1[0:32, :], dtype=np.float32)
    hi = w1.astype(ml_dtypes.bfloat16)
    mid = (w1 - hi.astype(np.float32)).astype(ml_dtypes.bfloat16)
    lo = (w1 - hi.astype(np.float32) - mid.astype(np.float32)).astype(
        ml_dtypes.bfloat16
    )
    w1b = np.zeros((128, 128), dtype=ml_dtypes.bfloat16)
    w1b[0:32] = hi
    w1b[32:64] = mid
    w1b[64:96] = lo

    thr = (0.5 - np.asarray(b2[:NBITS], dtype=np.float32))  # [26]
    trep = np.tile(thr, 12)[None, :].repeat(128, axis=0)    # [128, 312]

    wvec = np.zeros(312, dtype=np.float32)
    for c in range(12):
        for h in range(2):
            for i in range(13):
                wvec[26 * c + 13 * h + i] = float(1 << i)
    wrep = wvec[None, :].repeat(128, axis=0).astype(ml_dtypes.bfloat16)

    cst = np.zeros((128, C_TOT), dtype=np.float32)
    cst[:, C_W1:C_W1 + 64] = np.ascontiguousarray(w1b).view(np.float32)
    cst[:, C_B1] = np.asarray(b1, dtype=np.float32)
    cst[:, C_W2:C_W2 + NBITS] = np.asarray(W2[:, :NBITS], dtype=np.float32)
    cst[:, C_TR:C_TR + 312] = trep
    cst[:, C_WR:C_WR + 156] = np.ascontiguousarray(wrep).view(np.float32)
    return {"cst": cst}


def make_bit_planes(virtual_addr):
    """Per-core [96, PER] bf16 0/1 bit planes (3x replicated)."""
    import ml_dtypes

    va32 = np.asarray(virtual_addr).astype(np.uint32)
    out = []
    for c in range(va32.size // PER):
        seg = va32[c * PER:(c + 1) * PER]
        bits = np.unpackbits(
            seg.view(np.uint8).reshape(-1, 4), axis=-1, bitorder="little"
        )  # [PER, 32]
        u16 = (bits.T.astype(np.uint16) * 0x3F80)  # [32, PER] bf16 bit pattern
        full = np.concatenate([u16, u16, u16], axis=0)  # [96, PER]
        out.append(np.ascontiguousarray(full).view(ml_dtypes.bfloat16))
    return out


def combine_output(o):
    """[NOUT, 128, OUTW] f32 -> [PER] int64."""
    res = np.empty(PER, dtype=np.int64)
    for t in range(N_ITERS):
        nch = SIZES[t] // MMCH
        off = OB_OFF[t]
        s = o[OB_OF[t], :, off:off + 2 * nch]       # [128, 2*nch]
        lo = s[:, 0::2].astype(np.int64)            # [128, nch]
        hi = s[:, 1::2].astype(np.int64)
        phys = (lo + 8192 * hi).T.reshape(-1)       # (chunk, partition) order
        res[CSTART[t]:CSTART[t + 1]] = phys
    return res


_NC_CACHE = {}
TRACE = False
LAST_RES = None


def kernel(virtual_addr, W1, b1, W2, b2):
    global LAST_RES
    if "nc" not in _NC_CACHE:
        nc = build_nc()
        nc.finalize()
        _NC_CACHE["nc"] = nc
    nc = _NC_CACHE["nc"]

    consts = make_const_inputs(W1, b1, W2, b2)
    planes = make_bit_planes(virtual_addr)
    in_maps = [{"bp": planes[c], **consts} for c in range(NCORES)]

    res = bass_utils.run_bass_kernel_spmd(
        nc, in_maps, list(range(NCORES)), trace=TRACE
    )
    LAST_RES = res

    outs = [combine_output(res.results[c]["outp"]) for c in range(NCORES)]
    return np.concatenate(outs)
